# revision 2
# baseline (speedup 1.0000x reference)
"""BiDAF attention-flow kernel for Trainium2 (8 NeuronCores, data-parallel).

Self-contained: hardcodes shapes B,C,Q,H2 = 64,512,64,256; n_labels=2.
kernel(**inputs) takes full unsharded inputs, shards batch over 8 cores,
runs one SPMD Bass/Tile kernel per core, gathers [8,2] per core -> [64,2].

Per-core design (8 examples, bf16 compute):
  * fd loads CONTIGUOUSLY as R[k, (e, r, d)] with context row c = 4*k + r
    -> 4 KB DMA lines at full bus bandwidth, landing example-by-example so
    compute starts while later examples stream.  The (4k+r) permutation of
    c is absorbed downstream: row-softmax is row-local, max/min pools over
    c are order-invariant, and q2c contracts R against pm columns computed
    in the same permuted order.  w_sim loads as [6,128] (6 descriptors) and
    is transposed on-chip; wlab/blab ride the idle sync-engine HWDGE.
  * c_T[d, (e,dh,r,k)] via PE transposes; S[k, (r,s,j)] = c_T-chunk^T @
    (w_m*q^T + w_c) + ones*qw per example-pair; P = exp(S) unstabilized
    (|S| is O(1)); den/pm via bf16 fold trees; Pn = P * (1/den).
  * PT[(s,j), (r,k)] via PE transpose; c2q^T[d, (r,k)] = q-lhsT @ PT.
  * q2c via K=128/N=1 matmuls with R chunks as lhsT (pm unnormalized;
    the 1/sum(pm) scale is applied once at the end) - no extra transposes.
  * max/min pools are bf16 2x fold trees on DVE (the kernel's critical
    engine); PSUM->SBUF moves mostly ride ACT, with the pipeline-head
    copies on DVE to cut cross-engine latency; piece3 uses
    max(q2c*cmax, q2c*cmin); engines are interleaved per pair so the DVE
    stays saturated from ~14us (first c_T ready) to the end.
"""

import os
import sys

for _p in ("/opt/trn_rl_repo", "/opt/pypackages"):
    if os.path.isdir(_p) and _p not in sys.path:
        sys.path.insert(0, _p)

import numpy as np

import concourse.bass as bass
import concourse.bacc as bacc
import concourse.tile as tile
import concourse.mybir as mybir
from concourse.bass_utils import run_bass_kernel_spmd
from concourse.masks import make_identity
from concourse.tile_rust import add_dep_helper

F32 = mybir.dt.float32
BF16 = mybir.dt.bfloat16
AX = mybir.AxisListType
OP = mybir.AluOpType
AF = mybir.ActivationFunctionType

N_CORES = 8
B, C, Q, H2 = 64, 512, 64, 256
NL = 2
EX = B // N_CORES
R4 = 4
DH = H2 // 128
NPAIR = EX // 2


def _body(tc, ctx, fd, fq, wsim, wlab, blab, out):
    nc = tc.nc

    consts = ctx.enter_context(tc.tile_pool(name="consts", bufs=1))
    bigbuf = ctx.enter_context(tc.tile_pool(name="bigbuf", bufs=1))
    sb_small = ctx.enter_context(tc.tile_pool(name="small", bufs=1))
    p_pool = ctx.enter_context(tc.tile_pool(name="p", bufs=2))
    pn_pool = ctx.enter_context(tc.tile_pool(name="pn", bufs=2))
    pt_pool = ctx.enter_context(tc.tile_pool(name="pt", bufs=2))
    den_pool = ctx.enter_context(tc.tile_pool(name="den", bufs=2))
    scr_pool = ctx.enter_context(tc.tile_pool(name="scr", bufs=2))
    q2_pool = ctx.enter_context(tc.tile_pool(name="q2", bufs=2))

    ps_ct_pool = ctx.enter_context(tc.tile_pool(name="psct", bufs=2, space="PSUM"))
    ps_s_pool = ctx.enter_context(tc.tile_pool(name="pss", bufs=2, space="PSUM"))
    ps_c2q_pool = ctx.enter_context(tc.tile_pool(name="psc", bufs=2, space="PSUM"))
    ps_misc_pool = ctx.enter_context(tc.tile_pool(name="psm", bufs=2, space="PSUM"))

    # ---- small always-needed consts (cheap descriptors) ----
    ones_bf = consts.tile([1, 128], BF16)
    nc.vector.memset(ones_bf[0:1, :], 1.0)
    ones128_bf = consts.tile([128, 1], BF16)
    nc.vector.memset(ones128_bf[:, :], 1.0)
    ones_f32 = consts.tile([1, 128], F32)
    nc.vector.memset(ones_f32[0:1, :], 1.0)
    id_bf = consts.tile([128, 128], BF16)
    make_identity(nc, id_bf[:, :])
    id8_f32 = consts.tile([8, 8], F32)
    make_identity(nc, id8_f32[:, :])

    # ---- loads: w_sim compact (6 desc) first, then R0, R1, q, R2..R7.
    # wlab/blab ride the (idle) sync-engine HWDGE so their descriptor volume
    # never touches the gpsimd SWDGE that feeds the big cast-loads.
    R = bigbuf.tile([128, EX, R4, H2], BF16)
    q_dup = bigbuf.tile([128, EX, H2], BF16)
    w6 = consts.tile([6, 128], F32)            # row = t*2+dh
    ld = nc.gpsimd.dma_start(w6[:, :], wsim[:].rearrange("(x p) -> x p", x=6))

    def load_R(e, prev):
        l2 = nc.gpsimd.dma_start(
            R[:, e, :, :], fd[e, :, :].rearrange("(k r) d -> k r d", r=R4)
        )
        add_dep_helper(l2.ins, prev.ins, sync=False, reason="load order")
        return l2

    ld = load_R(0, ld)
    ld = load_R(1, ld)
    for half in range(2):
        l2 = nc.gpsimd.dma_start(
            q_dup[64 * half:64 * half + 64, :, :],
            fq[:, :, :].rearrange("e j d -> j e d"))
        add_dep_helper(l2.ins, ld.ins, sync=False, reason="load order")
        ld = l2
    for e in range(2, EX):
        ld = load_R(e, ld)

    b_sb = consts.tile([1, NL], F32)
    nc.sync.dma_start(b_sb[0:1, :], blab[:].rearrange("(o l) -> o l", o=1))
    wlab_sb = consts.tile([128, 4 * DH, NL], F32)   # chunk = piece*DH + dh
    nc.sync.dma_start(wlab_sb[:, :, :], wlab[:, :].rearrange("(c p) l -> p c l", p=128))

    # w6 [6, 128] -> w_sb [128, 6] via PE transpose (f32)
    w_ps = ps_misc_pool.tile([128, 8], F32, tag="misc")
    nc.tensor.matmul(
        w_ps[:, 0:6], w6[:, :], id8_f32[0:6, 0:6],
        is_transpose=True, start=True, stop=True,
    )
    w_sb = consts.tile([128, 6], F32)          # col = t*2+dh; t: 0=w_c 1=w_q 2=w_m
    nc.scalar.copy(w_sb[:, :], w_ps[:, 0:6])
    wq_bf = consts.tile([128, DH], BF16)
    nc.scalar.copy(wq_bf[:, :], w_sb[:, 2:4])

    # ---- persistent SBUF ----
    c_T = bigbuf.tile([128, EX, DH, 512], BF16)      # [d%128, e, dh, (r k)]
    c2q_sb = bigbuf.tile([128, EX, DH, 512], BF16)
    q_T_sb = sb_small.tile([128, EX * DH, 64], BF16)  # col = e*2+dh
    rhs_qm = sb_small.tile([128, EX, DH, 64], BF16)
    qw_sb = sb_small.tile([1, EX * 64], BF16)
    final_f = sb_small.tile([128, 4, DH, EX], F32)
    cmin_f = sb_small.tile([128, DH, EX], F32)
    q2cT_sb = sb_small.tile([128, EX, DH], F32)
    pm_all = sb_small.tile([128, NPAIR, R4, 2], BF16)
    out_sb = sb_small.tile([EX, NL], F32)

    def pe_transpose_group(dsts, srcs):
        first = None
        for dst, src in zip(dsts, srcs):
            mm = nc.tensor.matmul(
                dst, src, id_bf[0:src.shape[0], 0:src.shape[0]],
                is_transpose=True,
                start=(first is None), stop=(dst is dsts[-1]),
                skip_group_check=True,
            )
            if first is None:
                first = mm
            else:
                add_dep_helper(mm.ins, first.ins, sync=False, reason="bank order")
        return first

    def emit_ct(pair, dve_copies=False):
        """c transposes + PSUM->SBUF copies for one pair."""
        for s in range(2):
            e = 2 * pair + s
            for dh in range(DH):
                ct_ps = ps_ct_pool.tile([128, R4, 128], BF16, tag="ct")
                pe_transpose_group(
                    [ct_ps[:, r, :] for r in range(R4)],
                    [R[:, e, r, dh * 128:(dh + 1) * 128] for r in range(R4)],
                )
                if dve_copies and dh == 0:
                    nc.vector.tensor_copy(c_T[:, e, dh, :], ct_ps[:, :, :])
                else:
                    nc.scalar.copy(c_T[:, e, dh, :], ct_ps[:, :, :])

    def pool_tree(src, dst, op, width, tag):
        """src [128, G, width] bf16 -> dst [128, G] via 2x fold tree."""
        cur = src
        w = width
        lvl = 0
        while w > 32:
            nxt = scr_pool.tile(list(src.shape[:-1]) + [w // 2], BF16, tag=f"{tag}{lvl}")
            nc.vector.tensor_tensor(
                nxt[...], cur[..., 0:w // 2], cur[..., w // 2:w], op=op)
            cur = nxt
            w //= 2
            lvl += 1
        nc.vector.tensor_reduce(dst, cur[...], axis=AX.X, op=op)

    def pools_c(pair):
        eP2 = slice(2 * pair, 2 * pair + 2)
        cp = c_T[:, eP2, :, :]
        pool_tree(cp, fview[:, 0, :, eP2].rearrange("p dh e -> p e dh"), OP.max, 512, "p0")
        pool_tree(cp, cmin_f[:, :, eP2].rearrange("p dh e -> p e dh"), OP.min, 512, "pm")

    # pair0's transposes first: they only need R[0], R[1]
    emit_ct(0, dve_copies=True)

    # ---- q stage ----
    tq = ps_misc_pool.tile([128, EX * DH, 64], BF16, tag="misc")
    pe_transpose_group(
        [tq[:, e * DH + dh, :] for e in range(EX) for dh in range(DH)],
        [q_dup[0:64, e, dh * 128:(dh + 1) * 128] for e in range(EX) for dh in range(DH)],
    )
    nc.vector.tensor_copy(q_T_sb[:, :, :], tq[:, :, :])
    for dh in range(DH):
        nc.scalar.activation(
            rhs_qm[:, :, dh, :],
            tq[:, :, :].rearrange("p (e dhh) j -> p e dhh j", dhh=DH)[:, :, dh, :],
            AF.Identity,
            bias=w_sb[:, 0 + dh:1 + dh], scale=w_sb[:, 4 + dh:5 + dh],
        )
    ps_qw = ps_misc_pool.tile([1, 512], F32, tag="misc")
    for dh in range(DH):
        nc.tensor.matmul(
            ps_qw[0:1, :], wq_bf[:, dh:dh + 1],
            q_T_sb[:, :, :].rearrange("p (e dhh) j -> p e dhh j", dhh=DH)[:, :, dh, :],
            start=(dh == 0), stop=(dh == DH - 1),
        )
    nc.vector.tensor_copy(qw_sb[0:1, :], ps_qw[0:1, :])

    fview = final_f[:, :, :, :]
    pools_c(0)
    for pair in range(NPAIR):
        e0 = 2 * pair
        eP = slice(e0, e0 + 2)

        # -- S for both slots --
        ps_s = ps_s_pool.tile([128, R4, 2, 64], F32)
        first_mm = None
        for s in range(2):
            e = e0 + s
            for r in range(R4):
                for dh in range(DH):
                    mm = nc.tensor.matmul(
                        ps_s[:, r, s, :],
                        c_T[:, e, dh, r * 128:(r + 1) * 128],
                        rhs_qm[:, e, dh, :],
                        start=(first_mm is None), stop=False,
                        skip_group_check=True,
                    )
                    if first_mm is None:
                        first_mm = mm
                    else:
                        add_dep_helper(mm.ins, first_mm.ins, sync=False, reason="bank order")
        for r in range(R4):
            mm = nc.tensor.matmul(
                ps_s[:, r, :, :], ones_bf[0:1, :],
                qw_sb[0:1, e0 * 64:(e0 + 2) * 64],
                start=False, stop=(r == R4 - 1),
                skip_group_check=True,
            )
            add_dep_helper(mm.ins, first_mm.ins, sync=False, reason="bank order")

        # -- softmax --
        P = p_pool.tile([128, R4, 2, 64], BF16)
        nc.scalar.activation(P[:, :, :, :], ps_s[:, :, :, :], AF.Exp)
        den = den_pool.tile([128, R4, 2], F32, tag="den")
        da = den_pool.tile([128, R4, 2, 32], BF16, tag="da")
        nc.vector.tensor_tensor(da[:, :, :, :], P[:, :, :, 0:32], P[:, :, :, 32:64], op=OP.add)
        db = den_pool.tile([128, R4, 2, 16], BF16, tag="db")
        nc.vector.tensor_tensor(db[:, :, :, :], da[:, :, :, 0:16], da[:, :, :, 16:32], op=OP.add)
        nc.vector.tensor_reduce(den[:, :, :], db[:, :, :, :], axis=AX.X, op=OP.add)
        ma = den_pool.tile([128, R4, 2, 32], BF16, tag="ma")
        nc.vector.tensor_tensor(ma[:, :, :, :], P[:, :, :, 0:32], P[:, :, :, 32:64], op=OP.max)
        mb = den_pool.tile([128, R4, 2, 16], BF16, tag="mb")
        nc.vector.tensor_tensor(mb[:, :, :, :], ma[:, :, :, 0:16], ma[:, :, :, 16:32], op=OP.max)
        nc.vector.tensor_reduce(pm_all[:, pair, :, :], mb[:, :, :, :], axis=AX.X, op=OP.max)
        rden = den_pool.tile([128, R4, 2], F32, tag="rden")
        nc.vector.reciprocal(rden[:, :, :], den[:, :, :])
        Pn = pn_pool.tile([128, R4, 2, 64], BF16)
        nc.vector.tensor_tensor(
            Pn[:, :, :, :], P[:, :, :, :],
            rden[:, :, :].unsqueeze(3).broadcast_to([128, R4, 2, 64]), op=OP.mult)

        # -- PT + c2q --
        pt_ps = ps_misc_pool.tile([128, R4, 128], BF16, tag="misc")
        pe_transpose_group(
            [pt_ps[:, r, :] for r in range(R4)],
            [Pn[:, r, :, :] for r in range(R4)],
        )
        PT_sb = pt_pool.tile([128, R4, 128], BF16)
        nc.scalar.copy(PT_sb[:, :, :], pt_ps[:, :, :])

        # next pair's transposes + c-pools fill the wait for this pair's c2q
        if pair + 1 < NPAIR:
            emit_ct(pair + 1)
            pools_c(pair + 1)

        for s in range(2):
            e = e0 + s
            for dh in range(DH):
                ps_c2q = ps_c2q_pool.tile([128, 512], F32)
                nc.tensor.matmul(
                    ps_c2q[:, :],
                    q_dup[s * 64:s * 64 + 64, e, dh * 128:(dh + 1) * 128],
                    PT_sb[s * 64:s * 64 + 64, :, :],
                    start=True, stop=True,
                    tile_position=(s * 64, 0),
                )
                nc.scalar.copy(c2q_sb[:, e, dh, :], ps_c2q[:, :])

        # -- piece1 / piece2 pools (per pair) --
        qpair = c2q_sb[:, eP, :, :]
        cpair = c_T[:, eP, :, :]
        pool_tree(qpair, fview[:, 1, :, eP].rearrange("p dh e -> p e dh"), OP.max, 512, "p1")
        prod = scr_pool.tile([128, 2, DH, 512], BF16, tag="prod")
        nc.vector.tensor_tensor(prod[:, :, :, :], cpair, qpair, op=OP.mult)
        pool_tree(prod, fview[:, 2, :, eP].rearrange("p dh e -> p e dh"), OP.max, 512, "p2")

        # -- q2c (unnormalized; 1/sum applied once at the end) --
        ps_q2c = ps_misc_pool.tile([128, 512], F32, tag="misc")
        q2c0 = None
        for s in range(2):
            e = e0 + s
            for dh in range(DH):
                col = s * DH + dh
                for r in range(R4):
                    mm = nc.tensor.matmul(
                        ps_q2c[:, col:col + 1],
                        R[:, e, r, dh * 128:(dh + 1) * 128],
                        pm_all[:, pair, r, s:s + 1],
                        start=(q2c0 is None), stop=(s == 1 and dh == DH - 1 and r == R4 - 1),
                        skip_group_check=True,
                    )
                    if q2c0 is None:
                        q2c0 = mm
                    else:
                        add_dep_helper(mm.ins, q2c0.ins, sync=False, reason="bank order")
        nc.vector.tensor_copy(
            q2cT_sb[:, eP, :].rearrange("p e dh -> p (e dh)"), ps_q2c[:, 0:4])

    # -- normalize q2c once, then piece3 for all examples --
    ps_sm = ps_misc_pool.tile([128, 512], F32, tag="misc")
    nc.tensor.matmul(
        ps_sm[0:1, 0:32], ones128_bf[:, :],
        pm_all[:, :, :, :].rearrange("p pr r s -> p pr s r"),
        start=True, stop=True,
    )
    sumb = q2_pool.tile([1, EX], F32, tag="sumb")
    nc.vector.tensor_reduce(
        sumb[0:1, :],
        ps_sm[0:1, 0:32].rearrange("o (pr s r) -> o (pr s) r", r=R4, s=2),
        axis=AX.X, op=OP.add)
    recipb = q2_pool.tile([1, EX], F32, tag="recipb")
    nc.vector.reciprocal(recipb[0:1, :], sumb[0:1, :])
    ps_rb = ps_misc_pool.tile([128, 512], F32, tag="misc")
    nc.tensor.matmul(
        ps_rb[:, 0:EX], ones_f32[0:1, :], recipb[0:1, :],
        start=True, stop=True, skip_group_check=True,
    )
    q2n = q2_pool.tile([128, EX, DH], F32, tag="q2n")
    nc.vector.tensor_tensor(
        q2n[:, :, :], q2cT_sb[:, :, :],
        ps_rb[:, 0:EX].unsqueeze(2).broadcast_to([128, EX, DH]), op=OP.mult)

    s3a = q2_pool.tile([128, EX, DH], F32, tag="s3a")
    s3b = q2_pool.tile([128, EX, DH], F32, tag="s3b")
    nc.vector.tensor_tensor(
        s3a[:, :, :], q2n[:, :, :],
        fview[:, 0, :, :].rearrange("p dh e -> p e dh"), op=OP.mult)
    nc.vector.tensor_tensor(
        s3b[:, :, :], q2n[:, :, :],
        cmin_f[:, :, :].rearrange("p dh e -> p e dh"), op=OP.mult)
    nc.vector.tensor_tensor(
        fview[:, 3, :, :].rearrange("p dh e -> p e dh"),
        s3a[:, :, :], s3b[:, :, :], op=OP.max)

    # ---- final ----
    ps_out = ps_misc_pool.tile([128, 512], F32, tag="misc")
    for k in range(4 * DH):
        piece, dh = k // DH, k % DH
        nc.tensor.matmul(
            ps_out[0:EX, 0:NL], final_f[:, piece, dh, :], wlab_sb[:, k, :],
            start=(k == 0), stop=False, skip_group_check=True,
        )
    nc.tensor.matmul(
        ps_out[0:EX, 0:NL], ones_f32[0:1, 0:EX], b_sb[0:1, :],
        start=False, stop=True, skip_group_check=True,
    )
    nc.vector.tensor_copy(out_sb[:, :], ps_out[0:EX, 0:NL])
    nc.sync.dma_start(out[:, :], out_sb[:, :])


def build_nc():
    nc = bacc.Bacc("TRN2", target_bir_lowering=False, debug=False)
    fd = nc.dram_tensor("fd", [EX, C, H2], F32, kind="ExternalInput")
    fq = nc.dram_tensor("fq", [EX, Q, H2], F32, kind="ExternalInput")
    wsim = nc.dram_tensor("wsim", [3 * H2], F32, kind="ExternalInput")
    wlab = nc.dram_tensor("wlab", [4 * H2, NL], F32, kind="ExternalInput")
    blab = nc.dram_tensor("blab", [NL], F32, kind="ExternalInput")
    out = nc.dram_tensor("out", [EX, NL], F32, kind="ExternalOutput")

    from contextlib import ExitStack
    with tile.TileContext(nc) as tc:
        with ExitStack() as ctx:
            _body(tc, ctx, fd[:, :, :], fq[:, :, :], wsim[:], wlab[:, :], blab[:], out[:, :])
    nc.compile()
    return nc


_NC_CACHE = None


def run(inputs, trace=False):
    global _NC_CACHE
    if _NC_CACHE is None:
        _NC_CACHE = build_nc()
    nc = _NC_CACHE

    fd = np.ascontiguousarray(np.asarray(inputs["feature_document"], dtype=np.float32))
    fq = np.ascontiguousarray(np.asarray(inputs["feature_query"], dtype=np.float32))
    wsim = np.ascontiguousarray(np.asarray(inputs["w_sim"], dtype=np.float32))
    wlab = np.ascontiguousarray(np.asarray(inputs["w_label"], dtype=np.float32))
    blab = np.ascontiguousarray(np.asarray(inputs["b_label"], dtype=np.float32))

    in_maps = []
    for core in range(N_CORES):
        sl = slice(core * EX, (core + 1) * EX)
        in_maps.append({
            "fd": fd[sl], "fq": fq[sl],
            "wsim": wsim, "wlab": wlab, "blab": blab,
        })
    res = run_bass_kernel_spmd(nc, in_maps, list(range(N_CORES)), trace=trace)
    outs = np.concatenate([np.asarray(res.results[i]["out"]) for i in range(N_CORES)], axis=0)
    return outs.astype(np.float32), res


def kernel(**inputs):
    outs, _ = run(inputs, trace=False)
    return outs
